# revision 19
# baseline (speedup 1.0000x reference)
"""Trainium2 Bass kernel for the sparse_attention nn_Kernel problem.

Math (per sample, all derived from the reference):
  t1 = p1w * x ; t2 = roll(t1, 1, ch) ; t3_k = shift_{k-3}(t2) (zero-padded, w)
  A  = x2 @ t1^T                      (c x c)
  B_k = x2 @ t3_k^T                   (c x c), and A = B_3 rolled by +1 on j
  t7_k = (A - B_k)/sqrt(hw)
  S = sum_k t7_k
  G = sum_k t7_k @ t3_k               (c x hw)
  out = (S @ x2 + roll(G, +1 h, -1 w)) / sqrt(c*K)

Layout tricks:
  - width padded 56 -> 64 with zeros (3 left / 5 right) so every unfold shift
    s in [-3,3] is a free-dim AP offset that reads the stored zeros at row
    edges (exactly the zero-padded unfold semantics).
  - bmm1 contracts over p' = h*64+w on partitions: operands are built as
    (p', c) tensors via DMA-transpose (bf16 XBAR). The 7 shifts become column
    offsets of the transpose *source*.
  - bmm2 contracts over channels: rhs is the (c, p') tensor read at column
    offset s.
  - the final cyclic roll on G: out = roll(G) + S@x2 is computed as
    Q = G + invroll(S@x2) (using a pre-inverse-rolled copy of x as rhs),
    then out = roll(Q) applied in the SBUF->HBM DMAs.
  - scale beta = 1/(sqrt(hw)*sqrt(c*K)) folded into the x^T operand of bmm1.

Each of the 8 cores processes one sample of the batch (data parallel).
"""

import math

import numpy as np

C = 256
H = 56
W = 56
WP = 64  # padded width
PADL = 3
NPP = H * WP  # 3584 padded positions
NCH = NPP // 128  # 28 chunks of 128 partitions
K = 7
SLACK = 8  # extra zero columns on both sides of the t2 buffer
BETA = 1.0 / (math.sqrt(H * W) * math.sqrt(C * K))
N_CORES = 8

_CACHE = {}


def _build_nc():
    import concourse.bass as bass
    import concourse.mybir as mybir
    import concourse.tile as tile
    from concourse import bacc

    f32 = mybir.dt.float32
    bf16 = mybir.dt.bfloat16

    nc = bacc.Bacc("TRN2", target_bir_lowering=False, debug=False)

    # inputs are pre-cast to bf16 on the host: quarter the load traffic of
    # f32 and the on-chip pipeline is bf16 from the first op anyway.
    xin = nc.dram_tensor("x", [C, H, W], bf16, kind="ExternalInput").ap()
    pwin = nc.dram_tensor("p1w", [C, H, W], bf16, kind="ExternalInput").ap()
    out = nc.dram_tensor("out", [C, H, W], f32, kind="ExternalOutput").ap()

    sub = mybir.AluOpType.subtract
    mult = mybir.AluOpType.mult

    with tile.TileContext(nc) as tc:
        with (
            tc.tile_pool(name="f32big", bufs=2) as pf32,
            tc.tile_pool(name="bfbig", bufs=4) as pbf,
            tc.tile_pool(name="bfroll", bufs=2) as pbr,
            tc.tile_pool(name="ptrans", bufs=1) as pxT,
            tc.tile_pool(name="pt3", bufs=3) as pt3,
            tc.tile_pool(name="small", bufs=1) as psm,
            tc.tile_pool(name="dram", bufs=1, space="DRAM") as pdr,
            tc.tile_pool(name="ps", bufs=4, space="PSUM") as pps,
        ):
            # ---------------- prep: load + pad + mul (all bf16) -------------
            # contiguous bf16 loads; width padding done on-chip. The pad/mul
            # work is spread over vector+scalar+gpsimd so no single engine
            # serializes the lead-in.
            x_cp = []  # (c-chunk) [128, 56, 56] bf16, unpadded
            p_cp = []
            for cb in range(2):
                xt = pf32.tile([128, H, W], bf16, tag="xls")
                nc.sync.dma_start(xt[:], xin[cb * 128 : (cb + 1) * 128])
                x_cp.append(xt)
                pt = pf32.tile([128, H, W], bf16, tag="pls")
                nc.sync.dma_start(pt[:], pwin[cb * 128 : (cb + 1) * 128])
                p_cp.append(pt)

            t1buf = []
            x_bf = []
            for cb in range(2):
                xb = pbf.tile([128, NPP], bf16, tag="bfbig")
                x_bf.append(xb)
                tb = pbf.tile([128, 2 * SLACK + NPP], bf16, tag="bfbig")
                t1buf.append(tb)

            def t1view(cb):
                # [128, 56, 64] view of the data region
                return t1buf[cb][:, SLACK : SLACK + NPP].rearrange(
                    "p (h w) -> p h w", w=WP
                )

            xroll = []
            for cb in range(2):
                # x_bf = x in the padded [128, 56, 64] layout (the beta scale
                # rides the transpose identity instead)
                xb3 = x_bf[cb].rearrange("p (h w) -> p h w", w=WP)
                nc.vector.memset(xb3[:, :, 0:PADL], 0.0)
                nc.vector.memset(xb3[:, :, PADL + W : WP], 0.0)
                nc.scalar.copy(xb3[:, :, PADL : PADL + W], x_cp[cb][:])

                # t1buf[m, SLACK + 64h + 3 + w] = t1[m, h, w], zeros elsewhere
                # (m = raw channel; the reference's channel roll is handled on
                # the small c x c results)
                tv = t1view(cb)
                nc.vector.memset(t1buf[cb][:, 0:SLACK], 0.0)
                nc.vector.memset(t1buf[cb][:, SLACK + NPP :], 0.0)
                nc.vector.memset(tv[:, :, 0:PADL], 0.0)
                nc.vector.memset(tv[:, :, PADL + W : WP], 0.0)
                nc.vector.tensor_mul(
                    tv[:, :, PADL : PADL + W], x_cp[cb][:], p_cp[cb][:]
                )



            # ---------------- transposes ----------------
            # t3_3 via the DMA XBAR on sync (it gates the shift bounce, and
            # this keeps the sync sequencer chain short); xpT via the tensor
            # engine (regular matmuls against a beta-scaled identity), which
            # is idle during the whole lead-in. PSUM results are copied to
            # SBUF bf16 round-robin across vector/scalar/gpsimd.
            # t3_3[p'-part, chunk, m] = t1p[m, p']  (the s=0 transpose)
            xpT = pxT.tile([128, NCH, C], bf16, tag="xpT")
            t3_3 = pxT.tile([128, NCH, C], bf16, tag="t3base")
            nc.sync.dma_start_transpose(
                t3_3[:, :, 0:128], t1buf[0][:, SLACK : SLACK + NPP]
            )
            nc.sync.dma_start_transpose(
                t3_3[:, :, 128:256], t1buf[1][:, SLACK : SLACK + NPP]
            )

            I_beta = psm.tile([128, 128], bf16, tag="ibeta")
            nc.gpsimd.memset(I_beta[:], 0.0)
            nc.gpsimd.affine_select(
                out=I_beta[:],
                in_=I_beta[:],
                compare_op=mybir.AluOpType.not_equal,
                fill=BETA,
                base=0,
                pattern=[[-1, 128]],
                channel_multiplier=1,
            )
            # gpsimd cannot read PSUM, so the PSUM->SBUF copies alternate
            # between vector and scalar.
            ncp = 0
            for cb in range(2):
                for t in range(NCH):
                    tr = pps.tile([128, 128], mybir.dt.float32, tag="tr", bufs=4)
                    nc.tensor.matmul(
                        tr[:],
                        x_bf[cb][:, 128 * t : 128 * (t + 1)],
                        I_beta[:],
                        start=True,
                        stop=True,
                    )
                    dst = xpT[:, t, cb * 128 : (cb + 1) * 128]
                    if ncp % 2 == 0:
                        nc.vector.tensor_copy(out=dst, in_=tr[:])
                    else:
                        nc.scalar.copy(dst, tr[:])
                    ncp += 1

            # xroll[j, h', w'] = x[j, (h'+1)%H, (w'-1)%W]  (bf16, unscaled).
            # Emitted after the transpose copies: only bmm2's S-term needs it,
            # so it must not delay the lead-in work queued on vector.
            for cb in range(2):
                xr = pbr.tile([128, H, W], bf16, tag="bfroll")
                nc.vector.tensor_copy(
                    out=xr[:, 0:55, 1:W], in_=x_cp[cb][:, 1:56, 0 : W - 1]
                )
                nc.vector.tensor_copy(
                    out=xr[:, 0:55, 0:1], in_=x_cp[cb][:, 1:56, W - 1 : W]
                )
                nc.vector.tensor_copy(
                    out=xr[:, 55:56, 1:W], in_=x_cp[cb][:, 0:1, 0 : W - 1]
                )
                nc.vector.tensor_copy(
                    out=xr[:, 55:56, 0:1], in_=x_cp[cb][:, 0:1, W - 1 : W]
                )
                xroll.append(xr)

            # Partition-shifted copies of t3_3, bounced through DRAM. A DMA
            # whose DRAM-side AP is strided executes on a SINGLE DMA engine
            # (~27 GB/s); a fully contiguous DRAM AP fans out across all 16
            # engines (~430 GB/s). So store the bounce FLAT with 4 zero rows
            # on both ends: every transfer (1 write, 6 shifted reads) is then
            # one contiguous-DRAM AP. The zero border rows stand in for the
            # chunk-wraparound positions; those pair only with x's width-pad
            # zeros in bmm1, so their value is free (zero keeps sim finite).
            zeros_b = psm.tile([128, 4 * NCH * C // 128], bf16, tag="zeros_b")
            nc.vector.memset(zeros_b[:], 0.0)
            dram_t3 = pdr.tile([136, NCH * C], bf16)
            nc.sync.dma_start(dram_t3[0:4], zeros_b[:])
            nc.sync.dma_start(dram_t3[132:136], zeros_b[:])
            nc.sync.dma_start(dram_t3[4:132], t3_3[:])

            def make_t3(k):
                # t3k[p, c, :] holds p' = 128c + p with value t1p[m, p' + s]
                s = k - 3
                t3k = pt3.tile([128, NCH, C], bf16, tag="t3")
                nc.sync.dma_start(t3k[:], dram_t3[4 + s : 132 + s])
                return t3k

            # Prefetch all shifted copies now: reads 1-3 start as soon as the
            # bounce write lands; reads 4+ pace themselves on buffer reuse
            # (WAR semaphores against the consuming matmuls).
            t3k_pre = {k: make_t3(k) for k in (0, 1, 2, 4, 5, 6)}

            # ---------------- bmm1: P_k = C_k^T * beta ----------------
            # C_k[i,m] = sum_p x[i,p] * shift_s(t1[m])[p];  A = C_3 (s=0)
            # u_k^T[m,i] = A^T[m+1,i] - C_k^T[m,i]  (these are bmm2's lhsT)
            P3_sb = []
            for jb in range(2):
                p3 = pps.tile([128, C], mybir.dt.float32, tag="ps")
                for t in range(NCH):
                    nc.tensor.matmul(
                        p3[:],
                        t3_3[:, t, jb * 128 : (jb + 1) * 128],
                        xpT[:, t, :],
                        start=(t == 0),
                        stop=(t == NCH - 1),
                    )
                sb = psm.tile([128, C], f32, tag=f"p3sb{jb}")
                nc.vector.tensor_copy(out=sb[:], in_=p3[:])
                P3_sb.append(sb)

            # A_sb[jb][j] = P3_sb[(j+1) global]  (partition roll via DRAM
            # bounce; flat layout + duplicated row 0 so both reads are single
            # contiguous-DRAM APs). Routed via the scalar engine's DMA queue
            # so these don't queue behind the t3k prefetch enqueues on sync.
            dramA = pdr.tile([257, C], f32)
            for jb in range(2):
                nc.scalar.dma_start(dramA[jb * 128 : (jb + 1) * 128], P3_sb[jb][:])
            nc.scalar.dma_start(dramA[256:257], P3_sb[0][0:1, :])
            A_sb = []
            for jb in range(2):
                asb = psm.tile([128, C], f32, tag=f"asb{jb}")
                A_sb.append(asb)
                nc.scalar.dma_start(asb[:], dramA[jb * 128 + 1 : jb * 128 + 129])

            # t7 tiles (bf16) and S accumulation (f32)
            t7 = {}
            S_accB = []
            for jb in range(2):
                t7t = psm.tile([128, C], bf16, tag=f"t7_3_{jb}")
                nc.vector.tensor_tensor(t7t[:], A_sb[jb][:], P3_sb[jb][:], sub)
                t7[(3, jb)] = t7t
                sa = psm.tile([128, C], f32, tag=f"saccb{jb}")
                nc.vector.tensor_copy(out=sa[:], in_=P3_sb[jb][:])
                S_accB.append(sa)

            for k in (0, 1, 2, 4, 5, 6):
                t3k = t3k_pre[k]
                for jb in range(2):
                    pk = pps.tile([128, C], mybir.dt.float32, tag="ps")
                    for t in range(NCH):
                        nc.tensor.matmul(
                            pk[:],
                            t3k[:, t, jb * 128 : (jb + 1) * 128],
                            xpT[:, t, :],
                            start=(t == 0),
                            stop=(t == NCH - 1),
                        )
                    t7t = psm.tile([128, C], bf16, tag=f"t7_{k}_{jb}")
                    nc.vector.tensor_tensor(t7t[:], A_sb[jb][:], pk[:], sub)
                    t7[(k, jb)] = t7t
                    nc.vector.tensor_add(S_accB[jb][:], S_accB[jb][:], pk[:])

            # S^T[j,i] = 7*A^T[j,i] - W[(j-1)%C, i],  W = sum_k C_k^T = S_accB
            # W rolled down by one channel, via a contiguous-DRAM bounce
            # (row 1+g = W[g], row 0 duplicates W[255]) — the SBUF->SBUF
            # partition-shift alternative serializes on one DMA engine.
            dramW = pdr.tile([257, C], f32)
            nc.scalar.dma_start(dramW[1:129], S_accB[0][:])
            nc.scalar.dma_start(dramW[129:257], S_accB[1][:])
            nc.scalar.dma_start(dramW[0:1], S_accB[1][127:128, :])
            W_roll = []
            for jb in range(2):
                wr = psm.tile([128, C], f32, tag=f"wroll{jb}")
                nc.scalar.dma_start(wr[:], dramW[jb * 128 : jb * 128 + 128])
                W_roll.append(wr)
            S_bf = []
            for jb in range(2):
                st = psm.tile([128, C], bf16, tag=f"sbf{jb}")
                nc.vector.scalar_tensor_tensor(
                    st[:], P3_sb[jb][:], 7.0, W_roll[jb][:], mult, sub
                )
                S_bf.append(st)

            # ---------------- bmm2: Q = G + invroll(S @ x2) ----------------
            out_sb = []
            for ib in range(2):
                osb = pf32.tile([128, H, W], f32, tag="f32big")
                out_sb.append(osb)

            HT = 8  # h rows per out tile
            for ib in range(2):
                for h0 in range(0, H, HT):
                    q = pps.tile([128, HT * W], mybir.dt.float32, tag="ps")
                    qv = q.rearrange("p (h w) -> p h w", w=W)
                    n_mm = 0
                    for jb in range(2):
                        for k in range(K):
                            s = k - 3
                            rhs = t1view(jb)[:, h0 : h0 + HT, PADL + s : PADL + s + W]
                            nc.tensor.matmul(
                                qv[:],
                                t7[(k, jb)][:, ib * 128 : (ib + 1) * 128],
                                rhs,
                                start=(n_mm == 0),
                                stop=False,
                            )
                            n_mm += 1
                    for jb in range(2):
                        nc.tensor.matmul(
                            qv[:],
                            S_bf[jb][:, ib * 128 : (ib + 1) * 128],
                            xroll[jb][:, h0 : h0 + HT, :],
                            start=False,
                            stop=(jb == 1),
                        )
                    # roll while copying PSUM->SBUF:
                    # out[h, w] = Q[(h-1)%H, (w+1)%W]
                    def roll_copy(r0, r1, d0):
                        nc.vector.tensor_copy(
                            out=out_sb[ib][:, d0 : d0 + (r1 - r0), 0 : W - 1],
                            in_=qv[:, r0:r1, 1:W],
                        )
                        nc.vector.tensor_copy(
                            out=out_sb[ib][:, d0 : d0 + (r1 - r0), W - 1 : W],
                            in_=qv[:, r0:r1, 0:1],
                        )

                    if h0 + HT < H:
                        roll_copy(0, HT, h0 + 1)
                    else:
                        roll_copy(0, HT - 1, h0 + 1)
                        roll_copy(HT - 1, HT, 0)

            # ---------------- output (contiguous, already rolled) ----------
            for ib in range(2):
                nc.sync.dma_start(out[ib * 128 : (ib + 1) * 128], out_sb[ib][:])

    nc.compile()
    return nc


def _get_nc():
    if "nc" not in _CACHE:
        _CACHE["nc"] = _build_nc()
    return _CACHE["nc"]


def kernel(x: np.ndarray, p1w: np.ndarray) -> np.ndarray:
    import ml_dtypes

    from concourse.bass_utils import run_bass_kernel_spmd

    n = x.shape[0]
    assert n == N_CORES
    x = np.ascontiguousarray(np.asarray(x).astype(ml_dtypes.bfloat16))
    pw = np.ascontiguousarray(np.asarray(p1w)[0].astype(ml_dtypes.bfloat16))

    nc = _get_nc()
    in_maps = [{"x": x[i], "p1w": pw} for i in range(n)]
    res = run_bass_kernel_spmd(nc, in_maps, list(range(N_CORES)))
    outs = [res.results[i]["out"] for i in range(n)]
    return np.stack(outs, axis=0).astype(np.float32)



# revision 20
# speedup vs baseline: 1.0194x; 1.0194x over previous
"""Trainium2 Bass kernel for the sparse_attention nn_Kernel problem.

Math (per sample, all derived from the reference):
  t1 = p1w * x ; t2 = roll(t1, 1, ch) ; t3_k = shift_{k-3}(t2) (zero-padded, w)
  A  = x2 @ t1^T                      (c x c)
  B_k = x2 @ t3_k^T                   (c x c), and A = B_3 rolled by +1 on j
  t7_k = (A - B_k)/sqrt(hw)
  S = sum_k t7_k
  G = sum_k t7_k @ t3_k               (c x hw)
  out = (S @ x2 + roll(G, +1 h, -1 w)) / sqrt(c*K)

Layout tricks:
  - width padded 56 -> 64 with zeros (3 left / 5 right) so every unfold shift
    s in [-3,3] is a free-dim AP offset that reads the stored zeros at row
    edges (exactly the zero-padded unfold semantics).
  - bmm1 contracts over p' = h*64+w on partitions: operands are built as
    (p', c) tensors via DMA-transpose (bf16 XBAR). The 7 shifts become column
    offsets of the transpose *source*.
  - bmm2 contracts over channels: rhs is the (c, p') tensor read at column
    offset s.
  - the final cyclic roll on G: out = roll(G) + S@x2 is computed as
    Q = G + invroll(S@x2) (using a pre-inverse-rolled copy of x as rhs),
    then out = roll(Q) applied in the SBUF->HBM DMAs.
  - scale beta = 1/(sqrt(hw)*sqrt(c*K)) folded into the x^T operand of bmm1.

Each of the 8 cores processes one sample of the batch (data parallel).
"""

import math

import numpy as np

C = 256
H = 56
W = 56
WP = 64  # padded width
PADL = 3
NPP = H * WP  # 3584 padded positions
NCH = NPP // 128  # 28 chunks of 128 partitions
K = 7
SLACK = 8  # extra zero columns on both sides of the t2 buffer
BETA = 1.0 / (math.sqrt(H * W) * math.sqrt(C * K))
N_CORES = 8

_CACHE = {}


def _build_nc():
    import concourse.bass as bass
    import concourse.mybir as mybir
    import concourse.tile as tile
    from concourse import bacc

    f32 = mybir.dt.float32
    bf16 = mybir.dt.bfloat16

    nc = bacc.Bacc("TRN2", target_bir_lowering=False, debug=False)

    # inputs are pre-cast to bf16 on the host: quarter the load traffic of
    # f32 and the on-chip pipeline is bf16 from the first op anyway.
    xin = nc.dram_tensor("x", [C, H, W], bf16, kind="ExternalInput").ap()
    pwin = nc.dram_tensor("p1w", [C, H, W], bf16, kind="ExternalInput").ap()
    out = nc.dram_tensor("out", [C, H, W], f32, kind="ExternalOutput").ap()

    sub = mybir.AluOpType.subtract
    mult = mybir.AluOpType.mult

    with tile.TileContext(nc) as tc:
        with (
            tc.tile_pool(name="f32big", bufs=2) as pf32,
            tc.tile_pool(name="bfbig", bufs=4) as pbf,
            tc.tile_pool(name="bfroll", bufs=2) as pbr,
            tc.tile_pool(name="ptrans", bufs=1) as pxT,
            tc.tile_pool(name="pt3", bufs=3) as pt3,
            tc.tile_pool(name="small", bufs=1) as psm,
            tc.tile_pool(name="dram", bufs=1, space="DRAM") as pdr,
            tc.tile_pool(name="ps", bufs=4, space="PSUM") as pps,
        ):
            # ---------------- prep: load + pad + mul (all bf16) -------------
            # contiguous bf16 loads; width padding done on-chip. The pad/mul
            # work is spread over vector+scalar+gpsimd so no single engine
            # serializes the lead-in.
            x_cp = []  # (c-chunk) [128, 56, 56] bf16, unpadded
            p_cp = []
            for cb in range(2):
                xt = pf32.tile([128, H, W], bf16, tag="xls")
                nc.sync.dma_start(xt[:], xin[cb * 128 : (cb + 1) * 128])
                x_cp.append(xt)
                pt = pf32.tile([128, H, W], bf16, tag="pls")
                nc.sync.dma_start(pt[:], pwin[cb * 128 : (cb + 1) * 128])
                p_cp.append(pt)

            t1buf = []
            x_bf = []
            for cb in range(2):
                xb = pbf.tile([128, NPP], bf16, tag="bfbig")
                x_bf.append(xb)
                tb = pbf.tile([128, 2 * SLACK + NPP], bf16, tag="bfbig")
                t1buf.append(tb)

            def t1view(cb):
                # [128, 56, 64] view of the data region
                return t1buf[cb][:, SLACK : SLACK + NPP].rearrange(
                    "p (h w) -> p h w", w=WP
                )

            xroll = []
            for cb in range(2):
                # x_bf = x in the padded [128, 56, 64] layout (the beta scale
                # rides the transpose identity instead)
                xb3 = x_bf[cb].rearrange("p (h w) -> p h w", w=WP)
                nc.vector.memset(xb3[:, :, 0:PADL], 0.0)
                nc.vector.memset(xb3[:, :, PADL + W : WP], 0.0)
                nc.scalar.copy(xb3[:, :, PADL : PADL + W], x_cp[cb][:])

                # t1buf[m, SLACK + 64h + 3 + w] = t1[m, h, w], zeros elsewhere
                # (m = raw channel; the reference's channel roll is handled on
                # the small c x c results)
                tv = t1view(cb)
                nc.vector.memset(t1buf[cb][:, 0:SLACK], 0.0)
                nc.vector.memset(t1buf[cb][:, SLACK + NPP :], 0.0)
                nc.vector.memset(tv[:, :, 0:PADL], 0.0)
                nc.vector.memset(tv[:, :, PADL + W : WP], 0.0)
                nc.vector.tensor_mul(
                    tv[:, :, PADL : PADL + W], x_cp[cb][:], p_cp[cb][:]
                )



            # ---------------- transposes (tensor engine) ----------------
            # Both t3_3 and xpT via regular matmuls against an identity: the
            # PE is idle during the whole lead-in, and the DMA XBAR route
            # costs ~4.4us of sync sequencer per call plus a slow 256B-packet
            # drain that gates the shift bounce. t3_3 (which the bounce chains
            # off) goes first; the beta scale rides xpT's identity. gpsimd
            # cannot read PSUM, so the PSUM->SBUF copies alternate between
            # vector and scalar.
            # t3_3[p'-part, chunk, m] = t1p[m, p']  (the s=0 transpose)
            xpT = pxT.tile([128, NCH, C], bf16, tag="xpT")
            t3_3 = pxT.tile([128, NCH, C], bf16, tag="t3base")

            I_one = psm.tile([128, 128], bf16, tag="ione")
            nc.gpsimd.memset(I_one[:], 0.0)
            nc.gpsimd.affine_select(
                out=I_one[:],
                in_=I_one[:],
                compare_op=mybir.AluOpType.not_equal,
                fill=1.0,
                base=0,
                pattern=[[-1, 128]],
                channel_multiplier=1,
            )
            I_beta = psm.tile([128, 128], bf16, tag="ibeta")
            nc.gpsimd.memset(I_beta[:], 0.0)
            nc.gpsimd.affine_select(
                out=I_beta[:],
                in_=I_beta[:],
                compare_op=mybir.AluOpType.not_equal,
                fill=BETA,
                base=0,
                pattern=[[-1, 128]],
                channel_multiplier=1,
            )

            ncp = 0

            def pe_transpose(src, ident, dst):
                nonlocal ncp
                tr = pps.tile([128, 128], mybir.dt.float32, tag="tr", bufs=4)
                nc.tensor.matmul(tr[:], src, ident[:], start=True, stop=True)
                if ncp % 2 == 0:
                    nc.vector.tensor_copy(out=dst, in_=tr[:])
                else:
                    nc.scalar.copy(dst, tr[:])
                ncp += 1

            for cb in range(2):
                for t in range(NCH):
                    pe_transpose(
                        t1buf[cb][:, SLACK + 128 * t : SLACK + 128 * (t + 1)],
                        I_one,
                        t3_3[:, t, cb * 128 : (cb + 1) * 128],
                    )
            for cb in range(2):
                for t in range(NCH):
                    pe_transpose(
                        x_bf[cb][:, 128 * t : 128 * (t + 1)],
                        I_beta,
                        xpT[:, t, cb * 128 : (cb + 1) * 128],
                    )

            # xroll[j, h', w'] = x[j, (h'+1)%H, (w'-1)%W]  (bf16, unscaled).
            # Emitted after the transpose copies: only bmm2's S-term needs it,
            # so it must not delay the lead-in work queued on vector.
            for cb in range(2):
                xr = pbr.tile([128, H, W], bf16, tag="bfroll")
                nc.vector.tensor_copy(
                    out=xr[:, 0:55, 1:W], in_=x_cp[cb][:, 1:56, 0 : W - 1]
                )
                nc.vector.tensor_copy(
                    out=xr[:, 0:55, 0:1], in_=x_cp[cb][:, 1:56, W - 1 : W]
                )
                nc.vector.tensor_copy(
                    out=xr[:, 55:56, 1:W], in_=x_cp[cb][:, 0:1, 0 : W - 1]
                )
                nc.vector.tensor_copy(
                    out=xr[:, 55:56, 0:1], in_=x_cp[cb][:, 0:1, W - 1 : W]
                )
                xroll.append(xr)

            # Partition-shifted copies of t3_3, bounced through DRAM. A DMA
            # whose DRAM-side AP is strided executes on a SINGLE DMA engine
            # (~27 GB/s); a fully contiguous DRAM AP fans out across all 16
            # engines (~430 GB/s). So store the bounce FLAT with 4 zero rows
            # on both ends: every transfer (1 write, 6 shifted reads) is then
            # one contiguous-DRAM AP. The zero border rows stand in for the
            # chunk-wraparound positions; those pair only with x's width-pad
            # zeros in bmm1, so their value is free (zero keeps sim finite).
            zeros_b = psm.tile([128, 4 * NCH * C // 128], bf16, tag="zeros_b")
            nc.vector.memset(zeros_b[:], 0.0)
            dram_t3 = pdr.tile([136, NCH * C], bf16)
            nc.sync.dma_start(dram_t3[0:4], zeros_b[:])
            nc.sync.dma_start(dram_t3[132:136], zeros_b[:])
            nc.sync.dma_start(dram_t3[4:132], t3_3[:])

            def make_t3(k):
                # t3k[p, c, :] holds p' = 128c + p with value t1p[m, p' + s]
                s = k - 3
                t3k = pt3.tile([128, NCH, C], bf16, tag="t3")
                nc.sync.dma_start(t3k[:], dram_t3[4 + s : 132 + s])
                return t3k

            # Prefetch all shifted copies now: reads 1-3 start as soon as the
            # bounce write lands; reads 4+ pace themselves on buffer reuse
            # (WAR semaphores against the consuming matmuls).
            t3k_pre = {k: make_t3(k) for k in (0, 1, 2, 4, 5, 6)}

            # ---------------- bmm1: P_k = C_k^T * beta ----------------
            # C_k[i,m] = sum_p x[i,p] * shift_s(t1[m])[p];  A = C_3 (s=0)
            # u_k^T[m,i] = A^T[m+1,i] - C_k^T[m,i]  (these are bmm2's lhsT)
            P3_sb = []
            for jb in range(2):
                p3 = pps.tile([128, C], mybir.dt.float32, tag="ps")
                for t in range(NCH):
                    nc.tensor.matmul(
                        p3[:],
                        t3_3[:, t, jb * 128 : (jb + 1) * 128],
                        xpT[:, t, :],
                        start=(t == 0),
                        stop=(t == NCH - 1),
                    )
                sb = psm.tile([128, C], f32, tag=f"p3sb{jb}")
                nc.vector.tensor_copy(out=sb[:], in_=p3[:])
                P3_sb.append(sb)

            # A_sb[jb][j] = P3_sb[(j+1) global]  (partition roll via DRAM
            # bounce; flat layout + duplicated row 0 so both reads are single
            # contiguous-DRAM APs). Routed via the scalar engine's DMA queue
            # so these don't queue behind the t3k prefetch enqueues on sync.
            dramA = pdr.tile([257, C], f32)
            for jb in range(2):
                nc.scalar.dma_start(dramA[jb * 128 : (jb + 1) * 128], P3_sb[jb][:])
            nc.scalar.dma_start(dramA[256:257], P3_sb[0][0:1, :])
            A_sb = []
            for jb in range(2):
                asb = psm.tile([128, C], f32, tag=f"asb{jb}")
                A_sb.append(asb)
                nc.scalar.dma_start(asb[:], dramA[jb * 128 + 1 : jb * 128 + 129])

            # t7 tiles (bf16) and S accumulation (f32)
            t7 = {}
            S_accB = []
            for jb in range(2):
                t7t = psm.tile([128, C], bf16, tag=f"t7_3_{jb}")
                nc.vector.tensor_tensor(t7t[:], A_sb[jb][:], P3_sb[jb][:], sub)
                t7[(3, jb)] = t7t
                sa = psm.tile([128, C], f32, tag=f"saccb{jb}")
                nc.vector.tensor_copy(out=sa[:], in_=P3_sb[jb][:])
                S_accB.append(sa)

            for k in (0, 1, 2, 4, 5, 6):
                t3k = t3k_pre[k]
                for jb in range(2):
                    pk = pps.tile([128, C], mybir.dt.float32, tag="ps")
                    for t in range(NCH):
                        nc.tensor.matmul(
                            pk[:],
                            t3k[:, t, jb * 128 : (jb + 1) * 128],
                            xpT[:, t, :],
                            start=(t == 0),
                            stop=(t == NCH - 1),
                        )
                    t7t = psm.tile([128, C], bf16, tag=f"t7_{k}_{jb}")
                    nc.vector.tensor_tensor(t7t[:], A_sb[jb][:], pk[:], sub)
                    t7[(k, jb)] = t7t
                    nc.vector.tensor_add(S_accB[jb][:], S_accB[jb][:], pk[:])

            # S^T[j,i] = 7*A^T[j,i] - W[(j-1)%C, i],  W = sum_k C_k^T = S_accB
            # W rolled down by one channel, via a contiguous-DRAM bounce
            # (row 1+g = W[g], row 0 duplicates W[255]) — the SBUF->SBUF
            # partition-shift alternative serializes on one DMA engine.
            dramW = pdr.tile([257, C], f32)
            nc.scalar.dma_start(dramW[1:129], S_accB[0][:])
            nc.scalar.dma_start(dramW[129:257], S_accB[1][:])
            nc.scalar.dma_start(dramW[0:1], S_accB[1][127:128, :])
            W_roll = []
            for jb in range(2):
                wr = psm.tile([128, C], f32, tag=f"wroll{jb}")
                nc.scalar.dma_start(wr[:], dramW[jb * 128 : jb * 128 + 128])
                W_roll.append(wr)
            S_bf = []
            for jb in range(2):
                st = psm.tile([128, C], bf16, tag=f"sbf{jb}")
                nc.vector.scalar_tensor_tensor(
                    st[:], P3_sb[jb][:], 7.0, W_roll[jb][:], mult, sub
                )
                S_bf.append(st)

            # ---------------- bmm2: Q = G + invroll(S @ x2) ----------------
            out_sb = []
            for ib in range(2):
                osb = pf32.tile([128, H, W], f32, tag="f32big")
                out_sb.append(osb)

            HT = 8  # h rows per out tile
            for ib in range(2):
                for h0 in range(0, H, HT):
                    q = pps.tile([128, HT * W], mybir.dt.float32, tag="ps")
                    qv = q.rearrange("p (h w) -> p h w", w=W)
                    n_mm = 0
                    for jb in range(2):
                        for k in range(K):
                            s = k - 3
                            rhs = t1view(jb)[:, h0 : h0 + HT, PADL + s : PADL + s + W]
                            nc.tensor.matmul(
                                qv[:],
                                t7[(k, jb)][:, ib * 128 : (ib + 1) * 128],
                                rhs,
                                start=(n_mm == 0),
                                stop=False,
                            )
                            n_mm += 1
                    for jb in range(2):
                        nc.tensor.matmul(
                            qv[:],
                            S_bf[jb][:, ib * 128 : (ib + 1) * 128],
                            xroll[jb][:, h0 : h0 + HT, :],
                            start=False,
                            stop=(jb == 1),
                        )
                    # roll while copying PSUM->SBUF:
                    # out[h, w] = Q[(h-1)%H, (w+1)%W]
                    def roll_copy(r0, r1, d0):
                        nc.vector.tensor_copy(
                            out=out_sb[ib][:, d0 : d0 + (r1 - r0), 0 : W - 1],
                            in_=qv[:, r0:r1, 1:W],
                        )
                        nc.vector.tensor_copy(
                            out=out_sb[ib][:, d0 : d0 + (r1 - r0), W - 1 : W],
                            in_=qv[:, r0:r1, 0:1],
                        )

                    if h0 + HT < H:
                        roll_copy(0, HT, h0 + 1)
                    else:
                        roll_copy(0, HT - 1, h0 + 1)
                        roll_copy(HT - 1, HT, 0)

            # ---------------- output (contiguous, already rolled) ----------
            for ib in range(2):
                nc.sync.dma_start(out[ib * 128 : (ib + 1) * 128], out_sb[ib][:])

    nc.compile()
    return nc


def _get_nc():
    if "nc" not in _CACHE:
        _CACHE["nc"] = _build_nc()
    return _CACHE["nc"]


def kernel(x: np.ndarray, p1w: np.ndarray) -> np.ndarray:
    import ml_dtypes

    from concourse.bass_utils import run_bass_kernel_spmd

    n = x.shape[0]
    assert n == N_CORES
    x = np.ascontiguousarray(np.asarray(x).astype(ml_dtypes.bfloat16))
    pw = np.ascontiguousarray(np.asarray(p1w)[0].astype(ml_dtypes.bfloat16))

    nc = _get_nc()
    in_maps = [{"x": x[i], "p1w": pw} for i in range(n)]
    res = run_bass_kernel_spmd(nc, in_maps, list(range(N_CORES)))
    outs = [res.results[i]["out"] for i in range(n)]
    return np.stack(outs, axis=0).astype(np.float32)



# revision 22
# speedup vs baseline: 1.0593x; 1.0391x over previous
"""Trainium2 Bass kernel for the sparse_attention nn_Kernel problem.

Math (per sample, all derived from the reference):
  t1 = p1w * x ; t2 = roll(t1, 1, ch) ; t3_k = shift_{k-3}(t2) (zero-padded, w)
  A  = x2 @ t1^T                      (c x c)
  B_k = x2 @ t3_k^T                   (c x c), and A = B_3 rolled by +1 on j
  t7_k = (A - B_k)/sqrt(hw)
  S = sum_k t7_k
  G = sum_k t7_k @ t3_k               (c x hw)
  out = (S @ x2 + roll(G, +1 h, -1 w)) / sqrt(c*K)

Layout tricks:
  - width padded 56 -> 64 with zeros (3 left / 5 right) so every unfold shift
    s in [-3,3] is a free-dim AP offset that reads the stored zeros at row
    edges (exactly the zero-padded unfold semantics).
  - bmm1 contracts over p' = h*64+w on partitions: operands are built as
    (p', c) tensors via DMA-transpose (bf16 XBAR). The 7 shifts become column
    offsets of the transpose *source*.
  - bmm2 contracts over channels: rhs is the (c, p') tensor read at column
    offset s.
  - the final cyclic roll on G: out = roll(G) + S@x2 is computed as
    Q = G + invroll(S@x2) (using a pre-inverse-rolled copy of x as rhs),
    then out = roll(Q) applied in the SBUF->HBM DMAs.
  - scale beta = 1/(sqrt(hw)*sqrt(c*K)) folded into the x^T operand of bmm1.

Each of the 8 cores processes one sample of the batch (data parallel).
"""

import math

import numpy as np

C = 256
H = 56
W = 56
WP = 64  # padded width
PADL = 3
NPP = H * WP  # 3584 padded positions
NCH = NPP // 128  # 28 chunks of 128 partitions
K = 7
SLACK = 8  # extra zero columns on both sides of the t2 buffer
BETA = 1.0 / (math.sqrt(H * W) * math.sqrt(C * K))
N_CORES = 8

_CACHE = {}


def _build_nc():
    import concourse.bass as bass
    import concourse.mybir as mybir
    import concourse.tile as tile
    from concourse import bacc

    f32 = mybir.dt.float32
    bf16 = mybir.dt.bfloat16

    nc = bacc.Bacc("TRN2", target_bir_lowering=False, debug=False)

    # inputs are pre-cast to bf16 on the host: quarter the load traffic of
    # f32 and the on-chip pipeline is bf16 from the first op anyway.
    xin = nc.dram_tensor("x", [C, H, W], bf16, kind="ExternalInput").ap()
    pwin = nc.dram_tensor("p1w", [C, H, W], bf16, kind="ExternalInput").ap()
    out = nc.dram_tensor("out", [C, H, W], f32, kind="ExternalOutput").ap()

    sub = mybir.AluOpType.subtract
    mult = mybir.AluOpType.mult

    with tile.TileContext(nc) as tc:
        with (
            tc.tile_pool(name="f32big", bufs=2) as pf32,
            tc.tile_pool(name="bfbig", bufs=4) as pbf,
            tc.tile_pool(name="bfroll", bufs=2) as pbr,
            tc.tile_pool(name="ptrans", bufs=1) as pxT,
            tc.tile_pool(name="pt3", bufs=4) as pt3,
            tc.tile_pool(name="small", bufs=1) as psm,
            tc.tile_pool(name="dram", bufs=1, space="DRAM") as pdr,
            tc.tile_pool(name="ps", bufs=4, space="PSUM") as pps,
        ):
            # ---------------- prep: load + pad + mul (all bf16) -------------
            # contiguous bf16 loads; width padding done on-chip. The pad/mul
            # work is spread over vector+scalar+gpsimd so no single engine
            # serializes the lead-in.
            x_cp = []  # (c-chunk) [128, 56, 56] bf16, unpadded
            p_cp = []
            for cb in range(2):
                xt = pf32.tile([128, H, W], bf16, tag="xls")
                nc.sync.dma_start(xt[:], xin[cb * 128 : (cb + 1) * 128])
                x_cp.append(xt)
                pt = pf32.tile([128, H, W], bf16, tag="pls")
                nc.sync.dma_start(pt[:], pwin[cb * 128 : (cb + 1) * 128])
                p_cp.append(pt)

            t1buf = []
            x_bf = []
            for cb in range(2):
                xb = pbf.tile([128, NPP], bf16, tag="bfbig")
                x_bf.append(xb)
                tb = pbf.tile([128, 2 * SLACK + NPP], bf16, tag="bfbig")
                t1buf.append(tb)

            def t1view(cb):
                # [128, 56, 64] view of the data region
                return t1buf[cb][:, SLACK : SLACK + NPP].rearrange(
                    "p (h w) -> p h w", w=WP
                )

            xroll = []
            for cb in range(2):
                # x_bf = x in the padded [128, 56, 64] layout (the beta scale
                # rides the transpose identity instead)
                xb3 = x_bf[cb].rearrange("p (h w) -> p h w", w=WP)
                nc.vector.memset(xb3[:, :, 0:PADL], 0.0)
                nc.vector.memset(xb3[:, :, PADL + W : WP], 0.0)
                nc.scalar.copy(xb3[:, :, PADL : PADL + W], x_cp[cb][:])

                # t1buf[m, SLACK + 64h + 3 + w] = t1[m, h, w], zeros elsewhere
                # (m = raw channel; the reference's channel roll is handled on
                # the small c x c results)
                tv = t1view(cb)
                nc.vector.memset(t1buf[cb][:, 0:SLACK], 0.0)
                nc.vector.memset(t1buf[cb][:, SLACK + NPP :], 0.0)
                nc.vector.memset(tv[:, :, 0:PADL], 0.0)
                nc.vector.memset(tv[:, :, PADL + W : WP], 0.0)
                nc.vector.tensor_mul(
                    tv[:, :, PADL : PADL + W], x_cp[cb][:], p_cp[cb][:]
                )



            # ---------------- transposes (tensor engine) ----------------
            # Both t3_3 and xpT via regular matmuls against an identity: the
            # PE is idle during the whole lead-in, and the DMA XBAR route
            # costs ~4.4us of sync sequencer per call plus a slow 256B-packet
            # drain that gates the shift bounce. t3_3 (which the bounce chains
            # off) goes first; the beta scale rides xpT's identity. gpsimd
            # cannot read PSUM, so the PSUM->SBUF copies alternate between
            # vector and scalar.
            # t3_3[p'-part, chunk, m] = t1p[m, p']  (the s=0 transpose)
            xpT = pxT.tile([128, NCH, C], bf16, tag="xpT")
            t3_3 = pxT.tile([128, NCH, C], bf16, tag="t3base")

            I_one = psm.tile([128, 128], bf16, tag="ione")
            nc.gpsimd.memset(I_one[:], 0.0)
            nc.gpsimd.affine_select(
                out=I_one[:],
                in_=I_one[:],
                compare_op=mybir.AluOpType.not_equal,
                fill=1.0,
                base=0,
                pattern=[[-1, 128]],
                channel_multiplier=1,
            )
            I_beta = psm.tile([128, 128], bf16, tag="ibeta")
            nc.gpsimd.memset(I_beta[:], 0.0)
            nc.gpsimd.affine_select(
                out=I_beta[:],
                in_=I_beta[:],
                compare_op=mybir.AluOpType.not_equal,
                fill=BETA,
                base=0,
                pattern=[[-1, 128]],
                channel_multiplier=1,
            )

            # Four transpose chunks share one PSUM bank ([128, 512] f32, the
            # matmuls fill col-quarters under one accumulation group) and
            # drain with a single 4-chunk copy, so the PSUM->SBUF copies
            # don't pace the PE.
            ncp = 0

            def pe_transpose4(srcfn, ident, dstt):
                nonlocal ncp
                tr4 = pps.tile([128, 512], mybir.dt.float32, tag="tr", bufs=3)
                for i in range(4):
                    nc.tensor.matmul(
                        tr4[:, 128 * i : 128 * (i + 1)],
                        srcfn(i),
                        ident[:],
                        start=(i == 0),
                        stop=(i == 3),
                    )
                src = tr4.rearrange("p (c q) -> p c q", q=128)
                if ncp % 2 == 0:
                    nc.vector.tensor_copy(out=dstt, in_=src)
                else:
                    nc.scalar.copy(dstt, src)
                ncp += 1

            for cb in range(2):
                for tb in range(NCH // 4):
                    pe_transpose4(
                        lambda i, cb=cb, tb=tb: t1buf[cb][
                            :,
                            SLACK + 128 * (4 * tb + i) : SLACK + 128 * (4 * tb + i + 1),
                        ],
                        I_one,
                        t3_3[:, 4 * tb : 4 * tb + 4, cb * 128 : (cb + 1) * 128],
                    )
            for cb in range(2):
                for tb in range(NCH // 4):
                    pe_transpose4(
                        lambda i, cb=cb, tb=tb: x_bf[cb][
                            :, 128 * (4 * tb + i) : 128 * (4 * tb + i + 1)
                        ],
                        I_beta,
                        xpT[:, 4 * tb : 4 * tb + 4, cb * 128 : (cb + 1) * 128],
                    )

            # xroll[j, h', w'] = x[j, (h'+1)%H, (w'-1)%W]  (bf16, unscaled).
            # Emitted after the transpose copies: only bmm2's S-term needs it,
            # so it must not delay the lead-in work queued on vector.
            for cb in range(2):
                xr = pbr.tile([128, H, W], bf16, tag="bfroll")
                nc.vector.tensor_copy(
                    out=xr[:, 0:55, 1:W], in_=x_cp[cb][:, 1:56, 0 : W - 1]
                )
                nc.vector.tensor_copy(
                    out=xr[:, 0:55, 0:1], in_=x_cp[cb][:, 1:56, W - 1 : W]
                )
                nc.vector.tensor_copy(
                    out=xr[:, 55:56, 1:W], in_=x_cp[cb][:, 0:1, 0 : W - 1]
                )
                nc.vector.tensor_copy(
                    out=xr[:, 55:56, 0:1], in_=x_cp[cb][:, 0:1, W - 1 : W]
                )
                xroll.append(xr)

            # Partition-shifted copies of t3_3, bounced through DRAM. A DMA
            # whose DRAM-side AP is strided executes on a SINGLE DMA engine
            # (~27 GB/s); a fully contiguous DRAM AP fans out across all 16
            # engines (~430 GB/s). So store the bounce FLAT with 4 zero rows
            # on both ends: every transfer (1 write, 6 shifted reads) is then
            # one contiguous-DRAM AP. The zero border rows stand in for the
            # chunk-wraparound positions; those pair only with x's width-pad
            # zeros in bmm1, so their value is free (zero keeps sim finite).
            zeros_b = psm.tile([128, 4 * NCH * C // 128], bf16, tag="zeros_b")
            nc.vector.memset(zeros_b[:], 0.0)
            dram_t3 = pdr.tile([136, NCH * C], bf16)
            nc.sync.dma_start(dram_t3[0:4], zeros_b[:])
            nc.sync.dma_start(dram_t3[132:136], zeros_b[:])
            nc.sync.dma_start(dram_t3[4:132], t3_3[:])

            def make_t3(k):
                # t3k[p, c, :] holds p' = 128c + p with value t1p[m, p' + s]
                s = k - 3
                t3k = pt3.tile([128, NCH, C], bf16, tag="t3")
                nc.sync.dma_start(t3k[:], dram_t3[4 + s : 132 + s])
                return t3k

            # Prefetch all shifted copies now: reads 1-3 start as soon as the
            # bounce write lands; reads 4+ pace themselves on buffer reuse
            # (WAR semaphores against the consuming matmuls).
            t3k_pre = {k: make_t3(k) for k in (0, 1, 2, 4, 5, 6)}

            # ---------------- bmm1: P_k = C_k^T * beta ----------------
            # C_k[i,m] = sum_p x[i,p] * shift_s(t1[m])[p];  A = C_3 (s=0)
            # u_k^T[m,i] = A^T[m+1,i] - C_k^T[m,i]  (these are bmm2's lhsT)
            P3_sb = []
            for jb in range(2):
                p3 = pps.tile([128, C], mybir.dt.float32, tag="ps")
                for t in range(NCH):
                    nc.tensor.matmul(
                        p3[:],
                        t3_3[:, t, jb * 128 : (jb + 1) * 128],
                        xpT[:, t, :],
                        start=(t == 0),
                        stop=(t == NCH - 1),
                    )
                sb = psm.tile([128, C], f32, tag=f"p3sb{jb}")
                nc.vector.tensor_copy(out=sb[:], in_=p3[:])
                P3_sb.append(sb)

            # A_sb[jb][j] = P3_sb[(j+1) global]  (partition roll via DRAM
            # bounce; flat layout + duplicated row 0 so both reads are single
            # contiguous-DRAM APs). Routed via the scalar engine's DMA queue
            # so these don't queue behind the t3k prefetch enqueues on sync.
            dramA = pdr.tile([257, C], f32)
            for jb in range(2):
                nc.scalar.dma_start(dramA[jb * 128 : (jb + 1) * 128], P3_sb[jb][:])
            nc.scalar.dma_start(dramA[256:257], P3_sb[0][0:1, :])
            A_sb = []
            for jb in range(2):
                asb = psm.tile([128, C], f32, tag=f"asb{jb}")
                A_sb.append(asb)
                nc.scalar.dma_start(asb[:], dramA[jb * 128 + 1 : jb * 128 + 129])

            # t7 tiles (bf16) and S accumulation (f32)
            t7 = {}
            S_accB = []
            for jb in range(2):
                t7t = psm.tile([128, C], bf16, tag=f"t7_3_{jb}")
                nc.vector.tensor_tensor(t7t[:], A_sb[jb][:], P3_sb[jb][:], sub)
                t7[(3, jb)] = t7t
                sa = psm.tile([128, C], f32, tag=f"saccb{jb}")
                nc.vector.tensor_copy(out=sa[:], in_=P3_sb[jb][:])
                S_accB.append(sa)

            for k in (0, 1, 2, 4, 5, 6):
                t3k = t3k_pre[k]
                for jb in range(2):
                    pk = pps.tile([128, C], mybir.dt.float32, tag="ps")
                    for t in range(NCH):
                        nc.tensor.matmul(
                            pk[:],
                            t3k[:, t, jb * 128 : (jb + 1) * 128],
                            xpT[:, t, :],
                            start=(t == 0),
                            stop=(t == NCH - 1),
                        )
                    t7t = psm.tile([128, C], bf16, tag=f"t7_{k}_{jb}")
                    nc.vector.tensor_tensor(t7t[:], A_sb[jb][:], pk[:], sub)
                    t7[(k, jb)] = t7t
                    nc.vector.tensor_add(S_accB[jb][:], S_accB[jb][:], pk[:])

            # S^T[j,i] = 7*A^T[j,i] - W[(j-1)%C, i],  W = sum_k C_k^T = S_accB
            # W rolled down by one channel, via a contiguous-DRAM bounce
            # (row 1+g = W[g], row 0 duplicates W[255]) — the SBUF->SBUF
            # partition-shift alternative serializes on one DMA engine.
            dramW = pdr.tile([257, C], f32)
            nc.scalar.dma_start(dramW[1:129], S_accB[0][:])
            nc.scalar.dma_start(dramW[129:257], S_accB[1][:])
            nc.scalar.dma_start(dramW[0:1], S_accB[1][127:128, :])
            W_roll = []
            for jb in range(2):
                wr = psm.tile([128, C], f32, tag=f"wroll{jb}")
                nc.scalar.dma_start(wr[:], dramW[jb * 128 : jb * 128 + 128])
                W_roll.append(wr)
            S_bf = []
            for jb in range(2):
                st = psm.tile([128, C], bf16, tag=f"sbf{jb}")
                nc.vector.scalar_tensor_tensor(
                    st[:], P3_sb[jb][:], 7.0, W_roll[jb][:], mult, sub
                )
                S_bf.append(st)

            # ---------------- bmm2: Q = G + invroll(S @ x2) ----------------
            out_sb = []
            for ib in range(2):
                osb = pf32.tile([128, H, W], f32, tag="f32big")
                out_sb.append(osb)

            HT = 8  # h rows per out tile
            for ib in range(2):
                for h0 in range(0, H, HT):
                    q = pps.tile([128, HT * W], mybir.dt.float32, tag="ps")
                    qv = q.rearrange("p (h w) -> p h w", w=W)
                    n_mm = 0
                    for jb in range(2):
                        for k in range(K):
                            s = k - 3
                            rhs = t1view(jb)[:, h0 : h0 + HT, PADL + s : PADL + s + W]
                            nc.tensor.matmul(
                                qv[:],
                                t7[(k, jb)][:, ib * 128 : (ib + 1) * 128],
                                rhs,
                                start=(n_mm == 0),
                                stop=False,
                            )
                            n_mm += 1
                    for jb in range(2):
                        nc.tensor.matmul(
                            qv[:],
                            S_bf[jb][:, ib * 128 : (ib + 1) * 128],
                            xroll[jb][:, h0 : h0 + HT, :],
                            start=False,
                            stop=(jb == 1),
                        )
                    # roll while copying PSUM->SBUF:
                    # out[h, w] = Q[(h-1)%H, (w+1)%W]
                    def roll_copy(r0, r1, d0):
                        nc.vector.tensor_copy(
                            out=out_sb[ib][:, d0 : d0 + (r1 - r0), 0 : W - 1],
                            in_=qv[:, r0:r1, 1:W],
                        )
                        nc.vector.tensor_copy(
                            out=out_sb[ib][:, d0 : d0 + (r1 - r0), W - 1 : W],
                            in_=qv[:, r0:r1, 0:1],
                        )

                    if h0 + HT < H:
                        roll_copy(0, HT, h0 + 1)
                    else:
                        roll_copy(0, HT - 1, h0 + 1)
                        roll_copy(HT - 1, HT, 0)

            # ---------------- output (contiguous, already rolled) ----------
            for ib in range(2):
                nc.sync.dma_start(out[ib * 128 : (ib + 1) * 128], out_sb[ib][:])

    nc.compile()
    return nc


def _get_nc():
    if "nc" not in _CACHE:
        _CACHE["nc"] = _build_nc()
    return _CACHE["nc"]


def kernel(x: np.ndarray, p1w: np.ndarray) -> np.ndarray:
    import ml_dtypes

    from concourse.bass_utils import run_bass_kernel_spmd

    n = x.shape[0]
    assert n == N_CORES
    x = np.ascontiguousarray(np.asarray(x).astype(ml_dtypes.bfloat16))
    pw = np.ascontiguousarray(np.asarray(p1w)[0].astype(ml_dtypes.bfloat16))

    nc = _get_nc()
    in_maps = [{"x": x[i], "p1w": pw} for i in range(n)]
    res = run_bass_kernel_spmd(nc, in_maps, list(range(N_CORES)))
    outs = [res.results[i]["out"] for i in range(n)]
    return np.stack(outs, axis=0).astype(np.float32)

